# revision 1
# baseline (speedup 1.0000x reference)
"""Trainium2 Bass kernel for nn_CausalBankModel (V=32000, E=256, M=256, T=1024,
B=2, H=1024, W=8) on 8 NeuronCores.

Sharding: every core computes all B*T=2048 rows; the vocab dimension of both
readout W2 matrices (and the logits) is sharded 8 x 4000 (padded to 4096).
Vocab-axis stats (sum exp, sum l*exp(l), sum l, sum l^2, max) are computed
per-core over the shard and combined with AllReduce(add) + AllReduce(max); the
gate is computed redundantly on every core and applied to its logit shard.

Layout: rows n = b*T + t live on partitions (16 blocks of 128); vocab on the
free axis. The causal decaying state bank is a chunked scan: within a 128-step
chunk, states = diag(d^i) @ TriU @ diag(d^-j) via one 128x128 triangular
matmul per chunk per 128-mode half, plus a per-partition carry add (exact in
f32; worst-case rescale 0.85^-127 ~ 8.8e8 is well inside f32 range). Stats use
ScalarE activation accum_out (fused free-axis sum); max uses DVE reduce_max.
Logits are staged to HBM in bf16 between the stats pass and the gated mix.
"""

import sys

import numpy as np

sys.path.insert(0, "/opt/trn_rl_repo")

import ml_dtypes  # noqa: E402

from concourse import bacc, bass, mybir, tile  # noqa: E402
from concourse import bass_utils as _bu  # noqa: E402
from concourse.bass_utils import run_bass_kernel_spmd  # noqa: E402

# note: --enable-ldw-opt=true is incompatible with Bacc's
# move_matmul_waits_to_ldweights (walrus rejects the explicit InstLdweights),
# so LDWEIGHTS stays per-matmul.

F32 = mybir.dt.float32
BF16 = mybir.dt.bfloat16
I32 = mybir.dt.int32
AF = mybir.ActivationFunctionType
ALU = mybir.AluOpType
X_AXIS = mybir.AxisListType.X

V, E, M, T, B, H, W = 32000, 256, 256, 1024, 2, 1024, 8
N_CORES = 8
CORE_IDS = list(range(N_CORES))
NR = B * T            # 2048 rows
NBLK = NR // 128      # 16 row blocks
HBLK = H // 128       # 8 hidden blocks
VSH = V // N_CORES    # 4000 per-core vocab shard
VPAD = 4096           # padded shard width
VCH = VPAD // 512     # 8 v-chunks of 512
LPAD = T + W - 1      # 1031 padded columns per batch in x_T
CHUNK = 128           # scan chunk length
NCH = T // CHUNK      # 8 chunks per batch


def _bf(a):
    return np.ascontiguousarray(np.asarray(a).astype(ml_dtypes.bfloat16))


def build_program(decays_np, gate_w, gate_b, use_b2):
    """Build the per-core Bass program. decays / gate weights are baked into
    the NEFF as constants; everything else arrives via ExternalInputs."""
    import os
    kvar = os.environ.get("KVAR", "full")
    nc = bacc.Bacc(None, target_bir_lowering=False)

    emb_d = nc.dram_tensor("emb", [V, E], F32, kind="ExternalInput")
    chars_d = nc.dram_tensor("chars", [NBLK, 128, 1], I32, kind="ExternalInput")
    inp_d = nc.dram_tensor("in_proj", [E, M], F32, kind="ExternalInput")
    w1l_d = nc.dram_tensor("w1l", [M + E, H], BF16, kind="ExternalInput")
    w1o_d = nc.dram_tensor("w1o", [W * E, H], BF16, kind="ExternalInput")
    b1l_d = nc.dram_tensor("b1l", [HBLK, 128, 1], F32, kind="ExternalInput")
    b1o_d = nc.dram_tensor("b1o", [HBLK, 128, 1], F32, kind="ExternalInput")
    w2l_d = nc.dram_tensor("w2l", [H, VPAD], BF16, kind="ExternalInput")
    w2o_d = nc.dram_tensor("w2o", [H, VPAD], BF16, kind="ExternalInput")
    b2_d = nc.dram_tensor("b2", [2, 1, VPAD], BF16, kind="ExternalInput")
    out_d = nc.dram_tensor("out", [NR, VPAD], F32, kind="ExternalOutput")

    # host-precomputed scan constants baked into the NEFF
    j = np.arange(CHUNK)
    d64 = np.asarray(decays_np, dtype=np.float64)
    tri = np.triu(np.ones((CHUNK, CHUNK), np.float32))           # [j, i] j<=i
    dneg = (d64[None, :] ** (-j[:, None])).astype(np.float32)    # [j=128, M]
    dpow = (d64[:, None] ** j[None, :]).astype(np.float32)       # [M, i=128]
    tri_t = nc.inline_tensor(tri, "tri")
    dneg_t = nc.inline_tensor(dneg, "dneg")
    dpow_t = nc.inline_tensor(dpow, "dpow")
    ident_t = nc.inline_tensor(np.eye(128, dtype=np.float32), "ident")
    dvec_t = nc.inline_tensor(
        np.ascontiguousarray(d64.astype(np.float32).reshape(2, 128, 1)), "dvec"
    )

    def xcol(b, t):  # column of padded x_T for row n = b*T + t
        return b * LPAD + (W - 1) + t

    with tile.TileContext(nc) as tc, tile.ExitStack() as top:
        sb = top.enter_context(tc.tile_pool(name="sb", bufs=1))
        dr = top.enter_context(tc.tile_pool(name="dr", bufs=1, space="DRAM"))

        # ---------- resident sbuf state (lives for the whole kernel) ----------
        xtbf = [sb.tile([128, B * LPAD], BF16, tag=f"xtbf_{e}", name=f"xtbf_{e}") for e in range(2)]
        st_bf = [sb.tile([128, NR], BF16, tag=f"stbf_{m}", name=f"stbf_{m}") for m in range(2)]
        ones_s = sb.tile([1, 128], BF16, tag="ones1")
        nc.vector.memset(ones_s[:], 1.0)
        for e in range(2):
            nc.vector.memset(xtbf[e][:], 0.0)

        # per-path / global stat + staging state (kept in top-level pools)
        stats = {}  # (path, name) -> [128, NBLK] f32 tile
        for path in range(2):
            for name in ("se", "sa", "sl", "sq", "mx"):
                stats[(path, name)] = sb.tile([128, NBLK], F32, tag=f"st_{path}{name}", name=f"st_{path}{name}")
        ldram = [dr.tile([NBLK, 128, VPAD], BF16, name=f"ldram_{i}") for i in range(2)]

        # ---------- gather + transpose x + scan (scoped sbuf) ----------
        scn_cm = tile.ExitStack()
        scn = scn_cm.enter_context(tc.tile_pool(name="scn", bufs=1))
        xt32 = [scn.tile([128, B * LPAD], F32, tag=f"xt32_{e}", name=f"xt32_{e}") for e in range(2)]
        tri_s = scn.tile([128, 128], F32, tag="tri")
        dneg_s = scn.tile([128, M], F32, tag="dneg")
        dpow_s = [scn.tile([128, 128], F32, tag=f"dpow{m}", name=f"dpow{m}") for m in range(2)]
        ident_s = scn.tile([128, 128], F32, tag="ident")
        dvec_s = [scn.tile([128, 1], F32, tag=f"dvec{m}", name=f"dvec{m}") for m in range(2)]
        inp_s = scn.tile([128, 2, M], F32, tag="inp")
        nc.sync.dma_start(out=tri_s[:], in_=tri_t[:])
        nc.sync.dma_start(out=dneg_s[:], in_=dneg_t[:])
        for m in range(2):
            nc.sync.dma_start(out=dpow_s[m][:], in_=dpow_t[m * 128 : (m + 1) * 128, :])
            nc.sync.dma_start(out=dvec_s[m][:], in_=dvec_t[m])
        nc.sync.dma_start(out=ident_s[:], in_=ident_t[:])
        for e in range(2):
            nc.sync.dma_start(out=inp_s[:, e, :], in_=inp_d[e * 128 : (e + 1) * 128, :])
            nc.vector.memset(xt32[e][:], 0.0)
        with (
            tc.tile_pool(name="gat", bufs=3) as gat,
            tc.tile_pool(name="pt", bufs=4, space="PSUM") as pt,
        ):
            for k in range(NBLK):
                b, c = divmod(k, NCH)
                idx = gat.tile([128, 1], I32, tag="idx")
                nc.sync.dma_start(out=idx[:], in_=chars_d[k])
                xb = gat.tile([128, E], F32, tag="xb")
                if kvar == "fullnogather":
                    nc.sync.dma_start(out=xb[:], in_=emb_d[k * 128 : (k + 1) * 128, :])
                else:
                    nc.gpsimd.indirect_dma_start(
                        out=xb[:],
                        out_offset=None,
                        in_=emb_d[:],
                        in_offset=bass.IndirectOffsetOnAxis(ap=idx[:, :1], axis=0),
                    )
                col = xcol(b, c * CHUNK)
                for e in range(2):
                    ps = pt.tile([128, 128], F32, tag="ptr")
                    nc.tensor.transpose(
                        ps[:], xb[:, e * 128 : (e + 1) * 128], ident_s[:]
                    )
                    nc.scalar.copy(xt32[e][:, col : col + 128], ps[:])
                    nc.vector.tensor_copy(xtbf[e][:, col : col + 128], ps[:])

        # ---------- drive + causal decay scan ----------
        with (
            tc.tile_pool(name="cv", bufs=4) as cv,
            tc.tile_pool(name="pd", bufs=2, space="PSUM") as pd,
            tc.tile_pool(name="pc", bufs=4, space="PSUM") as pc,
        ):
            carry = {}
            for b in range(B):
                for m in range(2):
                    cz = cv.tile([128, 1], F32, tag=f"car{b}{m}")
                    nc.vector.memset(cz[:], 0.0)
                    carry[(b, m)] = cz
            for c in range(NCH):
                for b in range(B):
                    col = xcol(b, c * CHUNK)
                    psd = pd.tile([128, M], F32, tag="psd")
                    for e in range(2):
                        nc.tensor.matmul(
                            psd[:],
                            xt32[e][:, col : col + 128],
                            inp_s[:, e, :],
                            start=(e == 0),
                            stop=(e == 1),
                        )
                    scaled = cv.tile([128, M], F32, tag="scaled")
                    nc.vector.tensor_mul(scaled[:], psd[:], dneg_s[:])
                    n0 = b * T + c * CHUNK
                    for m in range(2):
                        psc = pc.tile([128, 128], F32, tag="psc")
                        nc.tensor.matmul(
                            psc[:],
                            scaled[:, m * 128 : (m + 1) * 128],
                            tri_s[:],
                            start=True,
                            stop=True,
                        )
                        # add decay-scaled carry (per-partition scalar)
                        nc.vector.tensor_scalar_add(psc[:], psc[:], carry[(b, m)][:])
                        # states_T (bf16) = psc * d^i
                        nc.vector.tensor_mul(
                            st_bf[m][:, n0 : n0 + CHUNK], psc[:], dpow_s[m][:]
                        )
                        # f32 carry for next chunk: d_m * (psc[:,127] * d^127)
                        cn = cv.tile([128, 1], F32, tag=f"car{b}{m}")
                        nc.vector.tensor_mul(
                            cn[:], psc[:, 127:128], dpow_s[m][:, 127:128]
                        )
                        nc.vector.tensor_mul(cn[:], cn[:], dvec_s[m][:])
                        carry[(b, m)] = cn
        scn_cm.close()

        # ---------- per-path: hidden layer, W2, stats ----------
        coll_out = {}
        coll_out_mx = {}
        paths = [] if kvar == "s1" else list(range(2))
        for path in paths:
            w1_d, b1_d, w2_dd = (
                (w1l_d, b1l_d, w2l_d) if path == 0 else (w1o_d, b1o_d, w2o_d)
            )
            nk1 = 4 if path == 0 else 16

            with tile.ExitStack() as ph:
                pp = ph.enter_context(tc.tile_pool(name=f"p{path}", bufs=1))
                ht = pp.tile([128, HBLK, NR], BF16, tag="ht")

                with (
                    tc.tile_pool(name=f"w1p{path}", bufs=1) as w1p,
                    tc.tile_pool(name=f"psh{path}", bufs=2, space="PSUM") as psh,
                ):
                    w1_s = w1p.tile([128, nk1, H], BF16, tag="w1")
                    for kk in range(nk1):
                        nc.sync.dma_start(
                            out=w1_s[:, kk, :], in_=w1_d[kk * 128 : (kk + 1) * 128, :]
                        )
                    b1_s = w1p.tile([128, HBLK], F32, tag="b1")
                    for hh in range(HBLK):
                        nc.sync.dma_start(out=b1_s[:, hh : hh + 1], in_=b1_d[hh])

                    def rhs_for(kk, q):
                        b, half = divmod(q, 2)
                        if path == 0:
                            if kk < 2:  # states rows of concat([states, x])
                                return st_bf[kk][:, q * 512 : (q + 1) * 512]
                            col = xcol(b, half * 512)
                            return xtbf[kk - 2][:, col : col + 512]
                        o, e = divmod(kk, 2)
                        col = xcol(b, half * 512) - o
                        return xtbf[e][:, col : col + 512]

                    for hh in range(HBLK):
                        psumhs = [
                            psh.tile([128, 512], F32, tag=f"ph{q}", bufs=2,
                                     name=f"ph{q}")
                            for q in range(4)
                        ]
                        for kk in range(nk1):
                            for q in range(4):
                                nc.tensor.matmul(
                                    psumhs[q][:],
                                    w1_s[:, kk, hh * 128 : (hh + 1) * 128],
                                    rhs_for(kk, q),
                                    start=(kk == 0),
                                    stop=(kk == nk1 - 1),
                                )
                        for q in range(4):
                            nc.scalar.activation(
                                ht[:, hh, q * 512 : (q + 1) * 512],
                                psumhs[q][:],
                                AF.Relu,
                                bias=b1_s[:, hh : hh + 1],
                            )

                # ---- W2 + stats ----
                w2_s = pp.tile([128, HBLK, VPAD], BF16, tag="w2")
                for hh in range(HBLK):
                    nc.sync.dma_start(
                        out=w2_s[:, hh, :], in_=w2_dd[hh * 128 : (hh + 1) * 128, :]
                    )
                b2_s = pp.tile([1, VPAD], BF16, tag="b2")
                if use_b2:
                    nc.sync.dma_start(out=b2_s[:], in_=b2_d[path])

                s_se = stats[(path, "se")]
                s_sa = stats[(path, "sa")]
                s_sl = stats[(path, "sl")]
                s_sq = stats[(path, "sq")]
                s_mx = stats[(path, "mx")]
                ld = ldram[path]

                if kvar == "s2":
                    continue
                with (
                    tc.tile_pool(name=f"stg{path}", bufs=3) as stp,
                    tc.tile_pool(name=f"pw{path}", bufs=2, space="PSUM") as pw,
                ):
                    for nb in range(NBLK):
                        stage = stp.tile([128, VPAD], BF16, tag="stage", bufs=2)
                        for vcg in range(2):
                            psls = [
                                pw.tile([128, 512], F32, tag=f"pl{i}", bufs=2,
                                        name=f"pl{i}")
                                for i in range(4)
                            ]
                            for hh in range(HBLK):
                                for i in range(4):
                                    vc = vcg * 4 + i
                                    nc.tensor.matmul(
                                        psls[i][:],
                                        ht[:, hh, nb * 128 : (nb + 1) * 128],
                                        w2_s[:, hh, vc * 512 : (vc + 1) * 512],
                                        start=(hh == 0),
                                        stop=(hh == HBLK - 1) and not use_b2,
                                    )
                            if use_b2:
                                for i in range(4):
                                    vc = vcg * 4 + i
                                    nc.tensor.matmul(
                                        psls[i][:],
                                        ones_s[:],
                                        b2_s[:, vc * 512 : (vc + 1) * 512],
                                        start=False,
                                        stop=True,
                                    )
                            for i in range(4):
                                vc = vcg * 4 + i
                                nc.vector.tensor_copy(
                                    stage[:, vc * 512 : (vc + 1) * 512],
                                    psls[i][:],
                                )
                        # sum-of-logits stat comes from the w2sum pad column
                        nc.vector.tensor_copy(
                            s_sl[:, nb : nb + 1], stage[:, VSH : VSH + 1]
                        )
                        if kvar == "s3":
                            nc.sync.dma_start(out=ld[nb], in_=stage[:])
                            continue
                        et = stp.tile([128, VSH], BF16, tag="et", bufs=2)
                        nc.scalar.activation(
                            et[:], stage[:, :VSH], AF.Exp,
                            accum_out=s_se[:, nb : nb + 1],
                        )
                        dump = stp.tile([128, VSH], BF16, tag="dump", bufs=1,
                                        name="dump")
                        nc.scalar.activation(
                            dump[:], stage[:, :VSH], AF.Square,
                            accum_out=s_sq[:, nb : nb + 1],
                        )
                        le = stp.tile([128, VSH], BF16, tag="le", bufs=2)
                        nc.vector.tensor_mul(le[:], stage[:, :VSH], et[:])
                        nc.scalar.activation(
                            dump[:], le[:], AF.Identity,
                            accum_out=s_sa[:, nb : nb + 1],
                        )
                        nc.vector.tensor_reduce(
                            s_mx[:, nb : nb + 1], stage[:, :VSH],
                            axis=X_AXIS, op=ALU.max,
                        )
                        nc.sync.dma_start(out=ld[nb], in_=stage[:])

                # per-path AllReduce issued immediately so path-0's collective
                # overlaps path-1 compute
                if kvar not in ("s2",):
                    cin = dr.tile([4, 128, NBLK], F32, name=f"cin{path}")
                    cout = dr.tile([4, 128, NBLK], F32, name=f"cout{path}")
                    cin_m = dr.tile([128, NBLK], F32, name=f"cinm{path}")
                    cout_m = dr.tile([128, NBLK], F32, name=f"coutm{path}")
                    for i, nm in enumerate(("se", "sa", "sl", "sq")):
                        nc.sync.dma_start(out=cin[i], in_=stats[(path, nm)][:])
                    nc.sync.dma_start(out=cin_m[:], in_=stats[(path, "mx")][:])
                    if kvar in ("fullnocoll", "s4", "s3"):
                        nc.sync.dma_start(out=cout[:], in_=cin[:])
                        nc.sync.dma_start(out=cout_m[:], in_=cin_m[:])
                    else:
                        nc.gpsimd.collective_compute(
                            "AllReduce", ALU.add, replica_groups=[CORE_IDS],
                            ins=[cin.opt()], outs=[cout.opt()],
                        )
                        nc.gpsimd.collective_compute(
                            "AllReduce", ALU.max, replica_groups=[CORE_IDS],
                            ins=[cin_m.opt()], outs=[cout_m.opt()],
                        )
                    coll_out[path] = cout
                    coll_out_mx[path] = cout_m

        # ---------- collective: AllReduce stats across the 8 cores ----------
        run_tail = kvar not in ("s1", "s2", "s3")
        if not run_tail:
            done = sb.tile([128, 16], F32, tag="done")
            nc.vector.memset(done[:], 1.0)
            nc.sync.dma_start(out=out_d[0:128, 0:16], in_=done[:])
        if run_tail:
            g_add = {}
            g_max = {}
            for p in range(2):
                for i, nm in enumerate(("se", "sa", "sl", "sq")):
                    t = sb.tile([128, NBLK], F32, tag=f"g_{p}{nm}",
                                name=f"g_{p}{nm}")
                    nc.sync.dma_start(out=t[:], in_=coll_out[p][i])
                    g_add[(p, nm)] = t
                t = sb.tile([128, NBLK], F32, tag=f"g_{p}mx", name=f"g_{p}mx")
                nc.sync.dma_start(out=t[:], in_=coll_out_mx[p][:])
                g_max[p] = t

            # ---------- gate from global stats (redundant on every core) ----------
            feats = []  # [ent_lin, mx_lin, var_lin, ent_loc, mx_loc, var_loc]
            with tc.tile_pool(name="gtp", bufs=1) as gtp:
                for p in range(2):
                    S, A = g_add[(p, "se")], g_add[(p, "sa")]
                    L, Q = g_add[(p, "sl")], g_add[(p, "sq")]
                    rS = gtp.tile([128, NBLK], F32, tag=f"rS{p}")
                    nc.vector.reciprocal(rS[:], S[:])
                    AoS = gtp.tile([128, NBLK], F32, tag=f"AoS{p}")
                    nc.vector.tensor_mul(AoS[:], A[:], rS[:])
                    lnS = gtp.tile([128, NBLK], F32, tag=f"lnS{p}")
                    nc.scalar.activation(lnS[:], S[:], AF.Ln)
                    ent = gtp.tile([128, NBLK], F32, tag=f"ent{p}")
                    nc.vector.tensor_sub(ent[:], lnS[:], AoS[:])
                    mean = gtp.tile([128, NBLK], F32, tag=f"mean{p}")
                    nc.vector.tensor_scalar_mul(mean[:], L[:], 1.0 / V)
                    m2 = gtp.tile([128, NBLK], F32, tag=f"m2{p}")
                    nc.vector.tensor_mul(m2[:], mean[:], mean[:])
                    var = gtp.tile([128, NBLK], F32, tag=f"var{p}")
                    nc.vector.tensor_scalar_mul(var[:], Q[:], 1.0 / V)
                    nc.vector.tensor_sub(var[:], var[:], m2[:])
                    feats += [ent, g_max[p], var]

                acc = gtp.tile([128, NBLK], F32, tag="gacc")
                nc.vector.tensor_scalar_mul(acc[:], feats[0][:], float(gate_w[0]))
                for i in range(1, 6):
                    nc.vector.scalar_tensor_tensor(
                        out=acc[:], in0=feats[i][:], scalar=float(gate_w[i]),
                        in1=acc[:], op0=ALU.mult, op1=ALU.add,
                    )
                gate = sb.tile([128, NBLK], F32, tag="gate")
                nc.scalar.activation(
                    gate[:], acc[:], AF.Sigmoid, bias=float(gate_b), scale=1.0
                )

            # ---------- gated mix ----------
            with tc.tile_pool(name="mx", bufs=3) as mxp:
                for nb in range(NBLK):
                    lin_s = mxp.tile([128, VPAD], BF16, tag="lin")
                    loc_s = mxp.tile([128, VPAD], BF16, tag="loc")
                    nc.sync.dma_start(out=lin_s[:], in_=ldram[0][nb])
                    nc.sync.dma_start(out=loc_s[:], in_=ldram[1][nb])
                    d = mxp.tile([128, VPAD], BF16, tag="d")
                    nc.vector.tensor_sub(d[:], lin_s[:], loc_s[:])
                    o = mxp.tile([128, VPAD], F32, tag="o")
                    nc.vector.scalar_tensor_tensor(
                        out=o[:], in0=d[:], scalar=gate[:, nb : nb + 1], in1=loc_s[:],
                        op0=ALU.mult, op1=ALU.add,
                    )
                    nc.sync.dma_start(
                    out=out_d[nb * 128 : (nb + 1) * 128, :VSH], in_=o[:, :VSH]
                )

    return _finish(nc)


def _finish(nc):
    nc.compile()
    return nc


def prepare_inputs(chars, emb, in_proj, lin_W1, lin_b1, lin_W2, lin_b2,
                   loc_W1, loc_b1, loc_W2, loc_b2):
    """Host-side shard/cast prep shared by all cores + per-core W2 shards."""
    chars_flat = np.asarray(chars).astype(np.int32).reshape(-1)
    chars_dev = np.ascontiguousarray(chars_flat.reshape(NBLK, 128, 1))

    common = dict(
        emb=np.ascontiguousarray(np.asarray(emb, np.float32)),
        chars=chars_dev,
        in_proj=np.ascontiguousarray(np.asarray(in_proj, np.float32)),
        w1l=_bf(lin_W1),
        w1o=_bf(loc_W1),
        b1l=np.ascontiguousarray(
            np.asarray(lin_b1, np.float32).reshape(HBLK, 128, 1)
        ),
        b1o=np.ascontiguousarray(
            np.asarray(loc_b1, np.float32).reshape(HBLK, 128, 1)
        ),
    )

    in_maps = []
    for c in range(N_CORES):
        sl = slice(c * VSH, (c + 1) * VSH)
        w2l = np.zeros((H, VPAD), np.float32)
        w2l[:, :VSH] = np.asarray(lin_W2, np.float32)[:, sl]
        w2o = np.zeros((H, VPAD), np.float32)
        w2o[:, :VSH] = np.asarray(loc_W2, np.float32)[:, sl]
        # pad column VSH = shard row-sum, so logits column VSH equals
        # sum_v l[n, v] (the sum-of-logits stat comes out of the matmul free)
        w2l[:, VSH] = np.asarray(lin_W2, np.float64)[:, sl].sum(axis=1)
        w2o[:, VSH] = np.asarray(loc_W2, np.float64)[:, sl].sum(axis=1)
        b2 = np.zeros((2, 1, VPAD), np.float32)
        b2[0, 0, :VSH] = np.asarray(lin_b2, np.float32)[sl]
        b2[1, 0, :VSH] = np.asarray(loc_b2, np.float32)[sl]
        b2[0, 0, VSH] = np.asarray(lin_b2, np.float64)[sl].sum()
        b2[1, 0, VSH] = np.asarray(loc_b2, np.float64)[sl].sum()
        in_maps.append(dict(common, w2l=_bf(w2l), w2o=_bf(w2o), b2=_bf(b2)))
    return in_maps


def assemble_output(results):
    parts = [results[c]["out"][:, :VSH] for c in range(N_CORES)]
    full = np.concatenate(parts, axis=1)
    return np.ascontiguousarray(full.reshape(B, T, V).astype(np.float32))


_CACHE = {}


def _get_program(decays, gate_W, gate_b, use_b2):
    key = (hash(np.asarray(decays, np.float64).tobytes()),
           hash(np.asarray(gate_W, np.float64).tobytes()),
           float(np.asarray(gate_b).reshape(-1)[0]), use_b2)
    if key not in _CACHE:
        _CACHE[key] = build_program(
            np.asarray(decays, np.float32),
            np.asarray(gate_W, np.float64).reshape(-1),
            float(np.asarray(gate_b).reshape(-1)[0]),
            use_b2,
        )
    return _CACHE[key]


def kernel(chars, emb, in_proj, decays, lin_W1, lin_b1, lin_W2, lin_b2,
           loc_W1, loc_b1, loc_W2, loc_b2, gate_W, gate_b):
    use_b2 = bool(np.any(np.asarray(lin_b2)) or np.any(np.asarray(loc_b2)))
    nc = _get_program(decays, gate_W, gate_b, use_b2)
    in_maps = prepare_inputs(chars, emb, in_proj, lin_W1, lin_b1, lin_W2,
                             lin_b2, loc_W1, loc_b1, loc_W2, loc_b2)
    res = run_bass_kernel_spmd(nc, in_maps, CORE_IDS)
    return assemble_output(res.results)

